# revision 1
# baseline (speedup 1.0000x reference)
"""Binarized dense layer (tanh(sign(x) @ sign(w) + b)) on 8 Trainium2 cores.

Full input shapes (hardcoded): inputs [8192, 4096] f32, kernel [4096, 4096] f32,
bias [4096] f32 -> out [8192, 4096] f32.

Sharding: 4 batch shards x 2 output-column shards (core i -> r=i//2, c=i%2).
Per core: x [2048, 4096], w [4096, 2048], b [2048] -> y [2048, 2048].

Wire format: inputs are shipped as bf16 (sign-preserving for all normal
floats; the binarize consumes only the sign, computed on device), and the
X shard is laid out K-major (transposed) so the contraction dim lands on
SBUF partitions directly. All reference ops (binarize X, binarize W,
matmul, bias add, tanh) run on device.

Per-core kernel (Tile framework):
  - W: DMA bf16 row-chunks, binarize to +-1 fp8e4 on ACT (Sign), resident
    in SBUF as [128, 2, O] per 256-row K-pair for fp8 DoubleRow matmul.
  - X^T: DMA bf16 k-chunks [128, B], binarize to +-0.5 fp8e4 with
    tensor_scalar (is_ge 0.0, subtract 0.5) on DVE/GpSimd; the whole
    binarized X^T (8.4 MB fp8) stays resident in SBUF.
  - Matmul: fp8 DoubleRow, K=256 per step, N=512 (one PSUM bank), M=128.
    PSUM accumulates 0.5 * (+-1 dot) exactly in f32.
  - Streaming phase: the first m_split m-tiles run k < KP/2 while the
    second halves of W/X^T stream in; partials spill to SBUF as fp16
    (exact: values are n/2 with |n/2| <= 1024) and are added back later.
  - Output: tanh on ACT reading PSUM with scale=2.0 (exact: psum = S/2),
    f32 staged in SBUF, DMA out.
"""

import sys
import types

if "/opt/trn_rl_repo" not in sys.path:
    sys.path.insert(0, "/opt/trn_rl_repo")

from contextlib import ExitStack

import numpy as np
import ml_dtypes

import concourse.bass as bass
import concourse.tile as tile
from concourse import bacc, mybir


def _ensure_ntff_hook_module():
    """The RL image's antenv lacks axon_hooks, which bass_utils imports for
    trace=True under axon. Register a functional shim in sys.modules."""
    name = "antenv.axon_hooks"
    if name in sys.modules:
        return
    try:
        import antenv
        __import__(name)
        return  # real module exists
    except ImportError:
        pass
    mod = types.ModuleType(name)
    mod._hook = None

    def set_axon_ntff_profile_hook(hook):
        mod._hook = hook

    def get_axon_ntff_profile_hook():
        if mod._hook is None:
            try:
                from trn_agent_boot.trn_boot import _ntff_profile_via_ctypes
                mod._hook = _ntff_profile_via_ctypes("/opt/axon/libaxon_pjrt.so")
            except Exception:
                return None
        return mod._hook

    mod.set_axon_ntff_profile_hook = set_axon_ntff_profile_hook
    mod.get_axon_ntff_profile_hook = get_axon_ntff_profile_hook
    sys.modules[name] = mod
    try:
        import antenv
        antenv.axon_hooks = mod
    except ImportError:
        pass


_ensure_ntff_hook_module()

from concourse.bass_utils import run_bass_kernel_spmd  # noqa: E402

F32 = mybir.dt.float32
F16 = mybir.dt.float16
BF16 = mybir.dt.bfloat16
FP8 = mybir.dt.float8e4

N_CORES = 8
R_SHARDS = 4  # batch shards
C_SHARDS = 2  # output-column shards

B_FULL, D_FULL, O_FULL = 8192, 4096, 4096
B_LOC = B_FULL // R_SHARDS   # 2048
O_LOC = O_FULL // C_SHARDS   # 2048


def build_nc(b_loc=B_LOC, d=D_FULL, o_loc=O_LOC, bias_nonzero=False,
             m_split=10, warmers=True):
    """Build the per-core Bass program (identical across cores)."""
    assert b_loc % 128 == 0 and d % 256 == 0 and o_loc % 512 == 0
    M = b_loc // 128    # m-tiles
    KP = d // 256       # DoubleRow K-pairs
    KC = d // 128       # 128-row chunks of the contraction dim
    N = o_loc // 512    # n-tiles (one PSUM bank each)
    m_split = min(m_split, M)
    if KP < 2:
        m_split = 0
    KH = KP // 2
    KCH = KC // 2

    nc = bacc.Bacc("TRN2", target_bir_lowering=False, debug=False,
                   num_devices=N_CORES)
    # x is the TRANSPOSED shard: [d, b_loc], K-major
    x = nc.dram_tensor("x", [d, b_loc], BF16, kind="ExternalInput")
    w = nc.dram_tensor("w", [d, o_loc], BF16, kind="ExternalInput")
    b = nc.dram_tensor("b", [o_loc], F32, kind="ExternalInput")
    y = nc.dram_tensor("y", [b_loc, o_loc], F32, kind="ExternalOutput")

    with tile.TileContext(nc) as tc, ExitStack() as ctx:
        singles = ctx.enter_context(tc.tile_pool(name="singles", bufs=1))
        wstage = ctx.enter_context(tc.tile_pool(name="wstage", bufs=3))
        wbp = ctx.enter_context(tc.tile_pool(name="wb", bufs=KP))
        xstage = ctx.enter_context(tc.tile_pool(name="xs", bufs=2))
        xbtp = ctx.enter_context(tc.tile_pool(name="xbt", bufs=1))
        ostage = ctx.enter_context(tc.tile_pool(name="ost", bufs=4))
        partp = ctx.enter_context(tc.tile_pool(name="part",
                                               bufs=max(m_split, 1)))
        pacc = ctx.enter_context(tc.tile_pool(name="pa", bufs=7,
                                              space="PSUM"))
        pscr = ctx.enter_context(tc.tile_pool(name="pscr", bufs=1,
                                              space="PSUM"))
        scratch = (pscr.tile([128, 64], F32, name="scratch")
                   if warmers else None)

        bias_bc = None
        if bias_nonzero:
            bias_bc = singles.tile([128, o_loc], F32)
            bias_ap = bass.AP(tensor=b.ap().tensor, offset=0,
                              ap=[[0, 128], [1, o_loc]])
            nc.gpsimd.dma_start(out=bias_bc[:], in_=bias_ap)
            # psum holds S/2 and tanh applies scale=2.0, so add bias/2
            nc.vector.tensor_scalar_mul(bias_bc[:], bias_bc[:], 0.5)

        # whole binarized X^T stays resident: [128, KC, b_loc] fp8
        xbt = xbtp.tile([128, KC, b_loc], FP8)

        def load_x_chunk(c, engine, dma=None):
            xs = xstage.tile([128, b_loc], BF16, tag="xs", name=f"xs{c}")
            (dma or nc.gpsimd).dma_start(out=xs[:],
                                         in_=x[c * 128:(c + 1) * 128, :])
            # binarize to +-0.5 fp8 in one pass
            engine.tensor_scalar(
                out=xbt[:, c, :], in0=xs[:], scalar1=0.0, scalar2=0.5,
                op0=mybir.AluOpType.is_ge, op1=mybir.AluOpType.subtract)

        def load_w_pair(k):
            t = wbp.tile([128, 2, o_loc], FP8, tag="wb", name=f"wb{k}")
            for j in (0, 1):
                s = wstage.tile([128, o_loc], BF16, tag="ws", name=f"ws{k}_{j}")
                nc.sync.dma_start(
                    out=s[:], in_=w[(2 * k + j) * 128:(2 * k + j + 1) * 128, :])
                nc.scalar.activation(out=t[:, j, :], in_=s[:],
                                     func=mybir.ActivationFunctionType.Sign)
            return t

        # ---- first halves of X^T and W (X on gpsimd queue, W on sync) ----
        for c in range(KCH):
            load_x_chunk(c, nc.vector)
        wb = []
        for k in range(KH):
            wb.append(load_w_pair(k))

        def k_group(pa, m, k0, k1, warm=False):
            for k in range(k0, k1):
                lhsT = xbt[:, 2 * k:2 * k + 2, m * 128:(m + 1) * 128]
                for n in range(N):
                    nc.tensor.matmul(
                        pa[n][:], lhsT, wb[k][:, :, n * 512:(n + 1) * 512],
                        start=(k == k0), stop=(k == k1 - 1),
                        perf_mode=mybir.MatmulPerfMode.DoubleRow)
                if warm and scratch is not None and k < k1 - 1:
                    # tiny matmul paced like the next X chunk: splits the
                    # W-arrival-paced idle gap below the ~3.4us HAM window
                    # so the PE keeps its 2.4 GHz clock. Same-arrival dep as
                    # the surrounding matmuls, so it adds no serialization.
                    c = min(2 * k + 1, KC - 1)
                    nc.tensor.matmul(scratch[:], xbt[:, c, 0:128],
                                     xbt[:, c, 0:64], start=True, stop=True)

        def finish(m, pa):
            o = ostage.tile([128, o_loc // 2], F32, tag="o", name=f"o{m}")
            for half in range(2):
                for nn in range(N // 2):
                    n = half * (N // 2) + nn
                    pn = pa[n][:]
                    if bias_bc is not None:
                        nc.vector.tensor_tensor(
                            out=pn, in0=pn,
                            in1=bias_bc[:, n * 512:(n + 1) * 512],
                            op=mybir.AluOpType.add)
                    nc.scalar.activation(
                        out=o[:, nn * 512:(nn + 1) * 512], in_=pn,
                        func=mybir.ActivationFunctionType.Tanh, scale=2.0)
                nc.sync.dma_start(
                    out=y[m * 128:(m + 1) * 128,
                          half * (o_loc // 2):(half + 1) * (o_loc // 2)],
                    in_=o[:])
                if half == 0:
                    o = ostage.tile([128, o_loc // 2], F32, tag="o",
                                    name=f"o{m}b")

        def alloc_pa(m):
            return [pacc.tile([128, 512], F32, tag="pa", name=f"pa_{m}_{n}")
                    for n in range(N)]

        # ---- phase 1: first m_split tiles accumulate k < KH while the
        # second halves stream in; partials spill to SBUF as fp16; the
        # second-half X binarizes are interleaved on the DVE queue so they
        # don't block behind (or get blocked by) the partial evictions ----
        parts = {}
        c2 = KCH  # next second-half X chunk to emit
        k2 = KH   # next second-half W pair to emit
        for m in range(m_split):
            pa = alloc_pa(m)
            k_group(pa, m, 0, KH, warm=(m == 0))
            part = partp.tile([128, N, 512], F16, tag="part", name=f"part{m}")
            for n in range(N):
                nc.vector.tensor_copy(out=part[:, n, :], in_=pa[n][:])
            parts[m] = part
            # interleave a slice of the second-half loads
            for _ in range((KH + m_split - 1) // m_split):
                if k2 < KP:
                    wb.append(load_w_pair(k2))
                    k2 += 1
            for _ in range((KCH + m_split - 1) // m_split):
                if c2 < KC:
                    load_x_chunk(c2, nc.vector, dma=nc.sync)
                    c2 += 1
        while k2 < KP:
            wb.append(load_w_pair(k2))
            k2 += 1
        while c2 < KC:
            load_x_chunk(c2, nc.vector, dma=nc.sync)
            c2 += 1

        # ---- phase 2: finish the split tiles (k >= KH, add partial) ----
        for m in range(m_split):
            pa = alloc_pa(m)
            k_group(pa, m, KH, KP)
            for n in range(N):
                nc.vector.tensor_tensor(out=pa[n][:], in0=pa[n][:],
                                        in1=parts[m][:, n, :],
                                        op=mybir.AluOpType.add)
            finish(m, pa)

        # ---- remaining m-tiles: single-pass k loop ----
        for m in range(m_split, M):
            pa = alloc_pa(m)
            k_group(pa, m, 0, KP)
            finish(m, pa)

    nc.compile()
    return nc


_NC_CACHE = {}


def _get_nc(key, **kwargs):
    if key not in _NC_CACHE:
        _NC_CACHE[key] = build_nc(**kwargs)
    return _NC_CACHE[key]


def kernel(inputs: np.ndarray, kernel: np.ndarray, bias: np.ndarray,
           _trace: bool = False, _trace_cores=None) -> np.ndarray:
    x = np.asarray(inputs, dtype=np.float32).astype(ml_dtypes.bfloat16)
    w = np.asarray(kernel, dtype=np.float32).astype(ml_dtypes.bfloat16)
    b = np.ascontiguousarray(bias, dtype=np.float32)
    assert x.shape == (B_FULL, D_FULL) and w.shape == (D_FULL, O_FULL)

    bias_nonzero = bool(np.any(b != 0))
    nc = _get_nc(("full", bias_nonzero), bias_nonzero=bias_nonzero,
                 m_split=8 if bias_nonzero else 10)

    in_maps = []
    for i in range(N_CORES):
        r, c = i // C_SHARDS, i % C_SHARDS
        in_maps.append({
            "x": np.ascontiguousarray(x[r * B_LOC:(r + 1) * B_LOC, :].T),
            "w": np.ascontiguousarray(w[:, c * O_LOC:(c + 1) * O_LOC]),
            "b": np.ascontiguousarray(b[c * O_LOC:(c + 1) * O_LOC]),
        })

    res = run_bass_kernel_spmd(nc, in_maps, list(range(N_CORES)),
                               trace=_trace, trace_cores=_trace_cores)

    out = np.empty((B_FULL, O_FULL), dtype=np.float32)
    for i in range(N_CORES):
        r, c = i // C_SHARDS, i % C_SHARDS
        out[r * B_LOC:(r + 1) * B_LOC, c * O_LOC:(c + 1) * O_LOC] = \
            res.results[i]["y"]

    if _trace:
        return out, res
    return out



# revision 3
# speedup vs baseline: 1.0007x; 1.0007x over previous
"""Binarized dense layer (tanh(sign(x) @ sign(w) + b)) on 8 Trainium2 cores, v2.

Full input shapes (hardcoded): inputs [8192, 4096] f32, kernel [4096, 4096] f32,
bias [4096] f32 -> out [8192, 4096] f32.

Sharding: 4 batch shards x 2 output-column shards (core i -> r=i//2, c=i%2).
Per core: x [2048, 4096], w [4096, 2048], b [2048] -> y [2048, 2048].

Wire format: both x (K-major, transposed) and w ship as fp8e4. fp8 conversion
preserves the f32 sign bit even when tiny values underflow to +-0, and the
on-device binarize extracts exactly that sign bit:
    byte' = (byte & 0x80) | 0x38     # 0x38 == fp8e4(+1.0)
done in-place on a uint32 view (4 bytes/lane/op) on the DVE. This is exact
wrt the reference binarize for every representable input, and makes all
matmul operands +-1 so PSUM accumulates the raw integer sum S (|S| <= 4096,
exact in f32); tanh needs no scale and f16 spills of half-K partials
(|S'| <= 2048) are exact.

The host additionally pre-interleaves w into the DoubleRow pair layout
(wire row k*128+p = [w[k*256+p, :], w[k*256+128+p, :]]) so a K-pair DMA is
a contiguous row slice with 4KB rows, and ships y back as bf16 (~7e-4
absmax rel err vs the 2e-2 gate) to halve output DMA.

Schedule (per core; M=16 m-tiles, KP=16 DoubleRow K-pairs, N=4 psum banks):
  - ~5.5us of dummy warm matmuls keep the PE HAM clock-gate busy while the
    first W pairs / X chunks stream in (PE hits 2.4 GHz before real work;
    the first DMA completions take ~5us regardless of size).
  - DMA priority order: W pair k (pairs 0/1 in column slices so the first
    matmul gates on a 0.125MB transfer) + X chunk col-halves (cols 0:1024,
    m-tiles 0-7) interleaved for k<KH, then remaining W pairs, then
    remaining X col-halves grouped 4 chunks/DMA, h1 halves last.
    W+out DMAs issue on the sync queue, X DMAs on the gpsimd queue.
  - Phase 1: m-tiles 0..m_split-1 in groups of 2 (8 psum banks), k<KH,
    k-inner so the two tiles' matmuls interleave and the W-pair demand
    rate (~0.43 MB/us) roughly matches HBM supply; partials spill to SBUF
    f16 (DVE and ACT split the copies so banks free fast).
  - Phase 2: same groups, k>=KH bank-major (each psum bank finishes its
    K-loop and evacuates - DVE partial add, ACT tanh scale=1, bf16 out
    DMA - under the next bank's matmuls, so banks recycle bubble-free).
  - m-tiles m_split..15 run single-pass full-K, also bank-major; the last
    tile DMAs out per-bank so the tail drains sooner.
"""

import sys
import types

if "/opt/trn_rl_repo" not in sys.path:
    sys.path.insert(0, "/opt/trn_rl_repo")

from contextlib import ExitStack

import numpy as np
import ml_dtypes

import concourse.bass as bass
import concourse.tile as tile
from concourse import bacc, mybir


def _ensure_ntff_hook_module():
    """The RL image's antenv lacks axon_hooks, which bass_utils imports for
    trace=True under axon. Register a functional shim in sys.modules."""
    name = "antenv.axon_hooks"
    if name in sys.modules:
        return
    try:
        import antenv
        __import__(name)
        return  # real module exists
    except ImportError:
        pass
    mod = types.ModuleType(name)
    mod._hook = None

    def set_axon_ntff_profile_hook(hook):
        mod._hook = hook

    def get_axon_ntff_profile_hook():
        if mod._hook is None:
            try:
                from trn_agent_boot.trn_boot import _ntff_profile_via_ctypes
                mod._hook = _ntff_profile_via_ctypes("/opt/axon/libaxon_pjrt.so")
            except Exception:
                return None
        return mod._hook

    mod.set_axon_ntff_profile_hook = set_axon_ntff_profile_hook
    mod.get_axon_ntff_profile_hook = get_axon_ntff_profile_hook
    sys.modules[name] = mod
    try:
        import antenv
        antenv.axon_hooks = mod
    except ImportError:
        pass


_ensure_ntff_hook_module()

from concourse.bass_utils import run_bass_kernel_spmd  # noqa: E402

F32 = mybir.dt.float32
F16 = mybir.dt.float16
BF16 = mybir.dt.bfloat16
FP8 = mybir.dt.float8e4
U32 = mybir.dt.uint32

N_CORES = 8
R_SHARDS = 4
C_SHARDS = 2

B_FULL, D_FULL, O_FULL = 8192, 4096, 4096
B_LOC = B_FULL // R_SHARDS   # 2048
O_LOC = O_FULL // C_SHARDS   # 2048

AND = mybir.AluOpType.bitwise_and
OR = mybir.AluOpType.bitwise_or
SIGN_MASK = 0x80808080
ONE_MASK = 0x38383838


def build_nc(b_loc=B_LOC, d=D_FULL, o_loc=O_LOC, bias_nonzero=False,
             out_f32=False, n_warm=62, m_split=8, kh=6):
    assert b_loc % 256 == 0 and d % 256 == 0 and o_loc % 1024 == 0
    M = b_loc // 128     # m-tiles
    KP = d // 256        # DoubleRow K-pairs
    KC = d // 128        # 128-row chunks of K
    N = o_loc // 512     # psum banks per m-tile
    BH = b_loc // 2      # x column-half (m-tiles 0..M/2-1)
    m_split = min(m_split, M)
    assert m_split % 2 == 0 and kh <= KP

    nc = bacc.Bacc("TRN2", target_bir_lowering=False, debug=False,
                   num_devices=N_CORES)
    x = nc.dram_tensor("x", [d, b_loc], FP8, kind="ExternalInput")
    # w ships pre-interleaved by the host: row k*128+p holds the K-pair k
    # DoubleRow layout [p, j, o] flattened, so a pair DMA is a plain row
    # slice with 4KB-contiguous rows.
    w = nc.dram_tensor("w", [KP * 128, 2 * o_loc], FP8, kind="ExternalInput")
    b = nc.dram_tensor("b", [o_loc], F32, kind="ExternalInput")
    out_dt = F32 if out_f32 else BF16
    y = nc.dram_tensor("y", [b_loc, o_loc], out_dt, kind="ExternalOutput")

    with tile.TileContext(nc) as tc, ExitStack() as ctx:
        singles = ctx.enter_context(tc.tile_pool(name="singles", bufs=1))
        partp = ctx.enter_context(tc.tile_pool(name="part",
                                               bufs=max(m_split, 1)))
        ostage = ctx.enter_context(tc.tile_pool(name="ost", bufs=4))
        pacc = ctx.enter_context(tc.tile_pool(name="pa", bufs=8,
                                              space="PSUM"))

        # ---- PE warmers: keep HAM busy while inputs stream in ----
        warmsrc = singles.tile([128, 128], FP8)
        nc.gpsimd.memset(warmsrc[:], 0)
        wpa = pacc.tile([128, 512], F32, tag="pa", name="warm")
        for _ in range(n_warm):
            nc.tensor.matmul(wpa[:, 0:128], warmsrc[:], warmsrc[:],
                             start=True, stop=True)

        # ---- residents ----
        xbt = singles.tile([128, KC, b_loc], FP8)
        wb = [singles.tile([128, 2, o_loc], FP8, name=f"wb{k}")
              for k in range(KP)]

        bias_bc = None
        if bias_nonzero:
            bias_bc = singles.tile([128, o_loc], F32)
            bias_ap = bass.AP(tensor=b.ap().tensor, offset=0,
                              ap=[[0, 128], [1, o_loc]])
            nc.sync.dma_start(out=bias_bc[:], in_=bias_ap)

        # ---- input DMAs + in-place binarize, in supply-priority order ----
        def binarize(sl):
            v = sl.bitcast(U32)
            nc.vector.tensor_scalar(out=v, in0=v, scalar1=SIGN_MASK,
                                    scalar2=ONE_MASK, op0=AND, op1=OR)

        # Binarize ops run on the DVE, whose queue is strict FIFO: a
        # binarize waiting on a late DMA would block every spill/add queued
        # behind it and stall PSUM recycling. So only the phase-1-critical
        # binarizes are emitted here; the rest are deferred (to points
        # between the phase bodies below) via `deferred`.
        deferred = []

        def dma_w_pair(k, n_slices=1, do_bin=True, dma_splits=1):
            # whole-pair DMAs (DMA-queue semaphore slots are scarce: ~8
            # per queue, recycled only after the consumer op runs) except
            # pair 0, which arrives in column halves so the very first
            # matmul gates on a 0.25MB transfer; the first pairs binarize
            # in column slices so the first matmuls gate on a 0.2us op
            dstep = o_loc // dma_splits
            for s in range(dma_splits):
                if dma_splits == 1:
                    nc.sync.dma_start(out=wb[k][:],
                                      in_=w[k * 128:(k + 1) * 128, :])
                else:
                    src = bass.AP(tensor=w.ap().tensor,
                                  offset=k * 128 * 2 * o_loc + s * dstep,
                                  ap=[[2 * o_loc, 128], [o_loc, 2],
                                      [1, dstep]])
                    nc.sync.dma_start(out=wb[k][:, :, s * dstep:(s + 1) * dstep],
                                      in_=src)
            step = o_loc // n_slices
            for s in range(n_slices):
                sl = wb[k][:, :, s * step:(s + 1) * step]
                if do_bin:
                    binarize(sl)
                else:
                    deferred.append(sl)

        def dma_x_h0(c):
            sl = xbt[:, c, 0:BH]
            nc.gpsimd.dma_start(out=sl, in_=x[c * 128:(c + 1) * 128, 0:BH])
            binarize(sl)

        def dma_x_group(c0, nch, h):
            # one DMA covering chunks [c0, c0+nch) cols of half h;
            # binarize deferred
            lo, hi = (0, BH) if h == 0 else (BH, b_loc)
            sl = xbt[:, c0:c0 + nch, lo:hi]
            src = bass.AP(tensor=x.ap().tensor, offset=c0 * 128 * b_loc + lo,
                          ap=[[b_loc, 128], [128 * b_loc, nch], [1, hi - lo]])
            nc.gpsimd.dma_start(out=sl, in_=src)
            deferred.append(sl)

        def flush_deferred(n):
            for sl in deferred[:n]:
                binarize(sl)
            del deferred[:n]

        for k in range(kh):
            dma_w_pair(k, n_slices=(4 if k == 0 else 2 if k == 1 else 1),
                       dma_splits=(2 if k == 0 else 1))
            dma_x_h0(2 * k)
            dma_x_h0(2 * k + 1)
        for k in range(kh, KP):
            dma_w_pair(k, do_bin=False)
        for c in range(2 * kh, KC, 4):
            dma_x_group(c, 4, 0)
        n_early_def = len(deferred)   # pairs kh.. + x-h0 groups
        for c in range(0, KC, 4):
            dma_x_group(c, 4, 1)
        n_h1_def = len(deferred) - n_early_def

        # ---- matmul machinery ----
        def alloc_banks(m):
            return [pacc.tile([128, 512], F32, tag="pa", name=f"pa_{m}_{n}")
                    for n in range(N)]

        def mm(pa, m, k, start, stop):
            lhsT = xbt[:, 2 * k:2 * k + 2, m * 128:(m + 1) * 128]
            for n in range(N):
                nc.tensor.matmul(
                    pa[n][:], lhsT, wb[k][:, :, n * 512:(n + 1) * 512],
                    start=start, stop=stop,
                    perf_mode=mybir.MatmulPerfMode.DoubleRow)

        def finish(m, pa, part, out_banks=2):
            o = ostage.tile([128, o_loc], out_dt, tag="o", name=f"o{m}")
            nb = N // out_banks   # banks per out DMA
            for n in range(N):
                if part is not None:
                    nc.vector.tensor_tensor(out=pa[n][:], in0=pa[n][:],
                                            in1=part[:, n, :],
                                            op=mybir.AluOpType.add)
                if bias_bc is not None:
                    nc.vector.tensor_tensor(
                        out=pa[n][:], in0=pa[n][:],
                        in1=bias_bc[:, n * 512:(n + 1) * 512],
                        op=mybir.AluOpType.add)
                nc.scalar.activation(
                    out=o[:, n * 512:(n + 1) * 512], in_=pa[n][:],
                    func=mybir.ActivationFunctionType.Tanh, scale=1.0)
                if (n + 1) % nb == 0:
                    lo, hi = (n + 1 - nb) * 512, (n + 1) * 512
                    nc.sync.dma_start(out=y[m * 128:(m + 1) * 128, lo:hi],
                                      in_=o[:, lo:hi])

        # ---- phase 1: m-pair groups, k < kh, spill f16 partials ----
        parts = {}
        for g in range(m_split // 2):
            ms = (2 * g, 2 * g + 1)
            pas = {m: alloc_banks(m) for m in ms}
            for k in range(kh):
                for m in ms:
                    mm(pas[m], m, k, start=(k == 0), stop=(k == kh - 1))
            for m in ms:
                part = partp.tile([128, N, 512], F16, tag="part",
                                  name=f"part{m}")
                for n in range(N):
                    # split spills DVE/ACT so banks free in parallel
                    if n % 2 == 0:
                        nc.vector.tensor_copy(out=part[:, n, :],
                                              in_=pas[m][n][:])
                    else:
                        nc.scalar.activation(
                            out=part[:, n, :], in_=pas[m][n][:],
                            func=mybir.ActivationFunctionType.Copy)
                parts[m] = part
            # late binarizes slot in behind this group's spills; their
            # DMAs have landed by now so they can't block the next group
            if g == 1:
                flush_deferred((n_early_def + 1) // 2)
            elif g == 2:
                flush_deferred(n_early_def // 2)

        def mm1(pa, m, k, n, start, stop):
            lhsT = xbt[:, 2 * k:2 * k + 2, m * 128:(m + 1) * 128]
            nc.tensor.matmul(
                pa[n][:], lhsT, wb[k][:, :, n * 512:(n + 1) * 512],
                start=start, stop=stop,
                perf_mode=mybir.MatmulPerfMode.DoubleRow)

        def evac_bank(m, pa, part, o, n, out_banks, split_last=False):
            if part is not None:
                nc.vector.tensor_tensor(out=pa[n][:], in0=pa[n][:],
                                        in1=part[:, n, :],
                                        op=mybir.AluOpType.add)
            if bias_bc is not None:
                nc.vector.tensor_tensor(
                    out=pa[n][:], in0=pa[n][:],
                    in1=bias_bc[:, n * 512:(n + 1) * 512],
                    op=mybir.AluOpType.add)
            if split_last:
                # final bank of the kernel: halve tanh+DMA so the out DMA
                # overlaps the second tanh and the tail drains sooner
                for h in range(2):
                    lo, hi = n * 512 + h * 256, n * 512 + (h + 1) * 256
                    nc.scalar.activation(
                        out=o[:, lo:hi], in_=pa[n][:, h * 256:(h + 1) * 256],
                        func=mybir.ActivationFunctionType.Tanh, scale=1.0)
                    nc.sync.dma_start(out=y[m * 128:(m + 1) * 128, lo:hi],
                                      in_=o[:, lo:hi])
                return
            nc.scalar.activation(
                out=o[:, n * 512:(n + 1) * 512], in_=pa[n][:],
                func=mybir.ActivationFunctionType.Tanh, scale=1.0)
            nb = N // out_banks
            if (n + 1) % nb == 0:
                lo, hi = (n + 1 - nb) * 512, (n + 1) * 512
                nc.sync.dma_start(out=y[m * 128:(m + 1) * 128, lo:hi],
                                  in_=o[:, lo:hi])

        # ---- phase 2: finish the split tiles (k >= kh, add partial) ----
        # bank-major: each psum bank's k-loop completes and evacuates under
        # the next bank's matmuls, so banks recycle without WAR bubbles
        for g in range(m_split // 2):
            ms = (2 * g, 2 * g + 1)
            pas = {m: alloc_banks(m) for m in ms}
            os_ = {m: ostage.tile([128, o_loc], out_dt, tag="o", name=f"o{m}")
                   for m in ms}
            for n in range(N):
                for m in ms:
                    for k in range(kh, KP):
                        mm1(pas[m], m, k, n, start=(k == kh),
                            stop=(k == KP - 1))
                for m in ms:
                    evac_bank(m, pas[m], parts[m], os_[m], n, out_banks=2)
            if g < 2:
                flush_deferred((n_h1_def + 1) // 2)  # x h1, for m-tiles 8+
        flush_deferred(len(deferred))

        # ---- remaining m-tiles: single-pass full K, bank-major ----
        for m in range(m_split, M):
            pa = alloc_banks(m)
            o = ostage.tile([128, o_loc], out_dt, tag="o", name=f"o{m}")
            ob = 4 if m == M - 1 else 2
            for n in range(N):
                for k in range(KP):
                    mm1(pa, m, k, n, start=(k == 0), stop=(k == KP - 1))
                evac_bank(m, pa, None, o, n, out_banks=ob,
                          split_last=(m == M - 1 and n == N - 1))

    nc.compile()
    return nc


_NC_CACHE = {}


def _get_nc(key, **kwargs):
    if key not in _NC_CACHE:
        _NC_CACHE[key] = build_nc(**kwargs)
    return _NC_CACHE[key]


def kernel(inputs: np.ndarray, kernel: np.ndarray, bias: np.ndarray,
           _trace: bool = False, _trace_cores=None, **_build_overrides) -> np.ndarray:
    x8 = np.asarray(inputs, dtype=np.float32).astype(ml_dtypes.float8_e4m3)
    w8 = np.asarray(kernel, dtype=np.float32).astype(ml_dtypes.float8_e4m3)
    b = np.ascontiguousarray(bias, dtype=np.float32)
    assert x8.shape == (B_FULL, D_FULL) and w8.shape == (D_FULL, O_FULL)

    bias_nonzero = bool(np.any(b != 0))
    key = ("v2", bias_nonzero, tuple(sorted(_build_overrides.items())))
    nc = _get_nc(key, bias_nonzero=bias_nonzero, **_build_overrides)

    in_maps = []
    for i in range(N_CORES):
        r, c = i // C_SHARDS, i % C_SHARDS
        # pre-interleave w for DoubleRow: wire row k*128+p = [w[k*256+p],
        # w[k*256+128+p]] concatenated -> pair DMA is a contiguous row slice
        wsh = w8[:, c * O_LOC:(c + 1) * O_LOC]
        wt = wsh.reshape(D_FULL // 256, 2, 128, O_LOC).transpose(0, 2, 1, 3)
        in_maps.append({
            "x": np.ascontiguousarray(x8[r * B_LOC:(r + 1) * B_LOC, :].T),
            "w": np.ascontiguousarray(wt).reshape(D_FULL // 2, 2 * O_LOC),
            "b": np.ascontiguousarray(b[c * O_LOC:(c + 1) * O_LOC]),
        })

    res = run_bass_kernel_spmd(nc, in_maps, list(range(N_CORES)),
                               trace=_trace, trace_cores=_trace_cores)

    out = np.empty((B_FULL, O_FULL), dtype=np.float32)
    for i in range(N_CORES):
        r, c = i // C_SHARDS, i % C_SHARDS
        out[r * B_LOC:(r + 1) * B_LOC, c * O_LOC:(c + 1) * O_LOC] = \
            np.asarray(res.results[i]["y"]).astype(np.float32)

    if _trace:
        return out, res
    return out
